# revision 1
# baseline (speedup 1.0000x reference)
"""Trainium2 Bass kernel for nn_AttentionLayer (B=4, C=64, N=4096, dk=64).

Math (per batch b):
    q_t[d, n] = (Wq/8) @ x[b]          # [64, N]
    k[d, m]   = Wk @ x[b]              # [64, N]
    v_t[n, o] = (Wv @ x[b]).T          # [N, 64]
    s[n, m]   = q_t.T @ k              # [N, N]
    attn      = softmax over n (columns)
    out[o, m] = v.T @ attn             # [64, N]

Sharding: 8 cores = 4 batches x 2 column-halves; core (b, h) computes
out[b, :, h*2048:(h+1)*2048]. The softmax axis n is fully local -> no
collectives. The tiny projections (0.25% of FLOPs) run on host so the
device inputs can be fed pre-laid-out in the matmul dtypes.

Device kernel per core (the N^2 part), fp16 compute (1 cyc/row on the PE;
float32r measures ~4 cyc/row on this hardware despite the cost model):
  - scores: TensorE fp16 matmuls [128x512] into [128, 1536] PSUM groups
    (3 banks, double-buffered so TensorE isn't WAR-blocked on ScalarE),
    lhsT = q_t chunk, rhs = k slice
  - exp: ScalarE straight out of PSUM -> fp16 SBUF (no max-subtraction:
    scores are O(+-6), fp32 PSUM and fp16 exp are safe)
  - AV: lhsT = v_t chunk [128, 65] fp16 (65th col = ones -> colsum),
    rhs = exp chunk, accumulated into PSUM [65, 512] f32 over 32 chunks
  - out DMA [65, 2048]: rows 0:64 = numerator, row 64 = colsum.
Host divides numerator by colsum and reassembles the full output.

Measured (loop-slope method, see bench.py): ~89.7 us/core on hardware,
rel_err 2.1e-04 vs the f64 reference.
"""

import ml_dtypes
import numpy as np

import concourse.bass as bass  # noqa: F401  (registers engine methods)
import concourse.mybir as mybir
import concourse.tile as tile
from concourse import bacc
from concourse.bass_utils import run_bass_kernel_spmd

B, C, N = 4, 64, 4096
MLOC = N // 2            # columns per core
P = 128
NCH = N // P             # 32 row-chunks of the score matrix
MT = 512                 # m-tile width (PSUM free dim)
NMT = MLOC // MT         # 4 m-tiles per core
GRP = 4                  # score chunks exp'd per ScalarE instruction
CP1 = C + 1              # v columns + ones column

F32 = mybir.dt.float32
F32R = mybir.dt.float32r
BF16 = mybir.dt.bfloat16
FP16 = mybir.dt.float16
EXP = mybir.ActivationFunctionType.Exp

_NC_CACHE = {}


def _build(grp=GRP, spsum_bufs=1, exp_bufs=2, prec="f32r", staged=False,
           dma_split=False, loop_reps=None):
    """Build the per-core graph.

    grp: score chunks per exp instruction ([128, grp*512] PSUM group).
    spsum_bufs: score-PSUM group buffers (grp*spsum_bufs + 2 <= 8 banks).
    exp_bufs: exp_sb SBUF buffers (32KB/partition each).
    prec: low-precision dtype for q/k (scores matmul) and v/exp (AV matmul).
        "f32r": q/k float32r, v/exp bf16. NOTE: float32r measures ~4 cyc/row
        on this hardware (the cost model wrongly says 1) -- do not ship.
        "bf16": all bf16 (1 cyc/row). "fp16": all float16 (1 cyc/row, 10
        mantissa bits -> ~8x less rounding error than bf16).
    staged: DVE-copy scores PSUM->SBUF half-tiles [128, 8192] and run exp
        from SBUF in 8 giant ScalarE instructions (grp ignored; PSUM =
        [128,1024]x3 + 2 AV banks). Targets slow ACT-from-PSUM reads.
    loop_reps: if set, wrap the attention body in a hardware For_i loop
        (used only for timing: per-iteration time = slope over reps).
    """
    if staged:
        grp, spsum_bufs = 2, 3
    assert grp * spsum_bufs + 2 <= 8
    qk_dt = {"f32r": F32R, "bf16": BF16, "fp16": FP16}[prec]
    lp_dt = {"f32r": BF16, "bf16": BF16, "fp16": FP16}[prec]
    nc = bacc.Bacc("TRN2", target_bir_lowering=False, debug=False)
    q_ext = nc.declare_dram_parameter("q", [C, N], qk_dt, isOutput=False)
    k_ext = nc.declare_dram_parameter("k", [C, MLOC], qk_dt, isOutput=False)
    v_ext = nc.declare_dram_parameter("v", [P, NCH * CP1], lp_dt, isOutput=False)
    out_ext = nc.declare_dram_parameter("out", [CP1, MLOC], F32, isOutput=True)

    # n-chunk groups per m-tile, e.g. grp=3 -> [3]*10 + [2]
    gsizes = []
    left = NCH
    while left > 0:
        gsizes.append(min(grp, left))
        left -= gsizes[-1]

    with tile.TileContext(nc) as tc:
        with (
            tc.tile_pool(name="const", bufs=1) as cpool,
            tc.tile_pool(name="expp", bufs=exp_bufs) as epool,
            tc.tile_pool(name="outp", bufs=2) as opool,
            tc.tile_pool(name="stg", bufs=2) as gpool,
            tc.tile_pool(name="spsum", bufs=spsum_bufs, space="PSUM") as spool,
            tc.tile_pool(name="apsum", bufs=2, space="PSUM") as apool,
        ):
            # One serial HWDGE queue -> emit in first-needed order: the first
            # scores group needs q[:, :384] and k[:, :512]; v is needed ~3us
            # in (first AV matmul); later k/q chunks are consumed much later.
            k_sb = cpool.tile([C, MLOC], qk_dt)
            q_sb = cpool.tile([C, N], qk_dt)
            v_sb = cpool.tile([P, NCH * CP1], lp_dt)
            vw = NCH * CP1 // 4

            def dq(j, eng=nc.sync):
                eng.dma_start(
                    q_sb[:, j * 512:(j + 1) * 512], q_ext[:, j * 512:(j + 1) * 512]
                )

            def dk(j, eng=nc.sync):
                eng.dma_start(
                    k_sb[:, j * 512:(j + 1) * 512], k_ext[:, j * 512:(j + 1) * 512]
                )

            def dv(j, eng=nc.sync):
                eng.dma_start(
                    v_sb[:, j * vw:(j + 1) * vw], v_ext[:, j * vw:(j + 1) * vw]
                )

            if dma_split:
                # two queues: sync(HWDGE) feeds the critical path (q, k0);
                # gpsimd(SWDGE) streams v and the k tail in parallel
                dq(0); dk(0); dq(1); dq(2); dq(3); dq(4); dq(5); dq(6); dq(7)
                for j in range(4):
                    dv(j, nc.gpsimd)
                for j in (1, 2, 3):
                    dk(j, nc.gpsimd)
            else:
                dq(0); dk(0); dq(1); dv(0); dq(2); dv(1); dq(3); dv(2)
                dq(4); dv(3); dq(5); dq(6); dq(7); dk(1); dk(2); dk(3)

            def q_ap(i):
                return q_sb[:, i * P:(i + 1) * P]

            def k_ap(t):
                return k_sb[:, t * MT:(t + 1) * MT]

            def attention_body(iv=None):
                for t in range(NMT):
                    exp_sb = epool.tile([P, NCH * MT], lp_dt, tag="exp")
                    if staged:
                        # 2 halves of 16 chunks: PE -> psum [128,1024] (2
                        # chunks) -> DVE copy -> s_half SBUF -> one giant exp
                        for h in range(2):
                            s_half = gpool.tile([P, 16 * MT], lp_dt, tag="sh")
                            for j in range(8):
                                ps = spool.tile([P, 2 * MT], F32, tag="sc")
                                for u in range(2):
                                    i = h * 16 + 2 * j + u
                                    nc.tensor.matmul(
                                        ps[:, u * MT:(u + 1) * MT],
                                        lhsT=q_ap(i),
                                        rhs=k_ap(t),
                                        start=True,
                                        stop=True,
                                    )
                                nc.vector.tensor_copy(
                                    s_half[:, j * 2 * MT:(j + 1) * 2 * MT], ps[:]
                                )
                            nc.scalar.activation(
                                exp_sb[:, h * 16 * MT:(h + 1) * 16 * MT],
                                s_half[:],
                                EXP,
                            )
                    else:
                        i = 0
                        for gs in gsizes:
                            ps = spool.tile([P, grp * MT], F32, tag="sc")
                            for u in range(gs):
                                nc.tensor.matmul(
                                    ps[:, u * MT:(u + 1) * MT],
                                    lhsT=q_ap(i + u),
                                    rhs=k_ap(t),
                                    start=True,
                                    stop=True,
                                )
                            nc.scalar.activation(
                                exp_sb[:, i * MT:(i + gs) * MT], ps[:, :gs * MT], EXP
                            )
                            i += gs
                    pav = apool.tile([CP1, MT], F32, tag="av")
                    for i in range(NCH):
                        nc.tensor.matmul(
                            pav[:],
                            lhsT=v_sb[:, i * CP1:(i + 1) * CP1],
                            rhs=exp_sb[:, i * MT:(i + 1) * MT],
                            start=(i == 0),
                            stop=(i == NCH - 1),
                        )
                    o_sb = opool.tile([CP1, MT], F32, tag="ot")
                    nc.vector.tensor_copy(o_sb[:], pav[:])
                    nc.sync.dma_start(out_ext[:, t * MT:(t + 1) * MT], o_sb[:])

            if loop_reps is None:
                attention_body()
            else:
                with tc.For_i(0, loop_reps, 1):
                    attention_body()

    nc.compile()
    return nc


BEST = {"grp": 3, "spsum_bufs": 2, "prec": "fp16"}


def _get_nc():
    if "nc" not in _NC_CACHE:
        _NC_CACHE["nc"] = _build(**BEST)
    return _NC_CACHE["nc"]


def _make_in_maps(x, Wq, Wk, Wv, prec="f32r"):
    qk_np = {"f32r": np.float32, "bf16": ml_dtypes.bfloat16, "fp16": np.float16}[prec]
    lp_np = {"f32r": ml_dtypes.bfloat16, "bf16": ml_dtypes.bfloat16,
             "fp16": np.float16}[prec]
    x = np.asarray(x, np.float32)
    wq8 = np.asarray(Wq, np.float32) * 0.125
    wk = np.asarray(Wk, np.float32)
    wv = np.asarray(Wv, np.float32)
    in_maps = []
    for b in range(B):
        xb = x[b]                                  # [C, N]
        qt = np.ascontiguousarray(wq8 @ xb)        # [C, N]
        kf = wk @ xb                               # [C, N]
        vt = (wv @ xb).T                           # [N, C]
        v3 = vt.reshape(NCH, P, C)
        va = np.concatenate([v3, np.ones((NCH, P, 1), np.float32)], axis=2)
        va = np.ascontiguousarray(
            va.transpose(1, 0, 2).reshape(P, NCH * CP1)
        ).astype(lp_np)
        for h in range(2):
            in_maps.append(
                {
                    "q": qt.astype(qk_np),
                    "k": np.ascontiguousarray(
                        kf[:, h * MLOC:(h + 1) * MLOC]
                    ).astype(qk_np),
                    "v": va,
                }
            )
    return in_maps


def _assemble(results):
    out = np.empty((B, C, N), np.float32)
    for core in range(2 * B):
        b, h = divmod(core, 2)
        r = results[core]["out"]
        out[b, :, h * MLOC:(h + 1) * MLOC] = r[:C] / r[C:C + 1]
    return out


def run(x, Wq, Wk, Wv, trace=False, **trace_kwargs):
    nc = _get_nc()
    res = run_bass_kernel_spmd(
        nc,
        _make_in_maps(x, Wq, Wk, Wv, prec=BEST.get("prec", "f32r")),
        core_ids=list(range(2 * B)),
        trace=trace,
        **trace_kwargs,
    )
    return _assemble(res.results), res


def kernel(x, Wq, Wk, Wv):
    out, _ = run(x, Wq, Wk, Wv, trace=False)
    return out



# revision 7
# speedup vs baseline: 7.4771x; 7.4771x over previous
"""Trainium2 Bass kernel for nn_AttentionLayer (B=4, C=64, N=4096, dk=64).

Math (per batch b):
    q_t[d, n] = (Wq/8) @ x[b]          # [64, N]
    k[d, m]   = Wk @ x[b]              # [64, N]
    v_t[n, o] = (Wv @ x[b]).T          # [N, 64]
    s[n, m]   = q_t.T @ k              # [N, N]
    attn      = softmax over n (columns)
    out[o, m] = v.T @ attn             # [64, N]

Sharding: 8 cores = 4 batches x 2 column-halves; core (b, h) computes
out[b, :, h*2048:(h+1)*2048]. The softmax axis n is fully local -> no
collectives. The tiny projections (0.25% of FLOPs) run on host so the
device inputs can be fed pre-laid-out in the matmul dtypes.

Device kernel per core (the N^2 part):
  - scores: TensorE fp16 matmuls [128x512] into [128, grp*512] PSUM
    groups (grp banks, double-buffered so TensorE isn't WAR-blocked)
  - exp: ScalarE straight out of PSUM, exp(s - ln4) via the free ACT
    bias (keeps e' <= ~66, under the TRN fp8e4 +-240 ceiling), written
    directly as fp8e4 into exp_sb [128, NCH, 512]
  - AV: fp8 DoubleRow pair-matmuls: lhsT = v pairs [128, 2, 65] (chunk
    stride padded to 80 B for the LDW step%16 rule, 65th col = ones ->
    colsum), rhs = exp pairs [128, 2, 512], accumulated into PSUM
    [65, 512] f32 over 16 pairs. DoubleRow feeds 2 fp8 contraction rows
    per cycle -> ~1.8x the fp16 AV rate.
  - AV pair-matmuls of m-tile t-1 are interleaved between the score
    groups of m-tile t so the PE never idles (HAM stays at K=8/8).
  - out DMA [65, 512] per m-tile: rows 0:64 = numerator, row 64 = colsum.
Host divides numerator by colsum and reassembles the full output.

PE work per core: scores 65536 cyc + AV ~37k cyc (vs 65536 fp16) at
2.4 GHz. rel_err ~7e-3 vs the f64 reference (fp8 quantization of v and
exp; gate is 2e-2). The all-fp16 path (prec="fp16", ~131072 cyc) is kept
for A/B.
"""

import ml_dtypes
import numpy as np

import concourse.bass as bass  # noqa: F401  (registers engine methods)
import concourse.mybir as mybir
import concourse.tile as tile
from concourse import bacc
from concourse.bass_utils import run_bass_kernel_spmd

B, C, N = 4, 64, 4096
MLOC = N // 2            # columns per core
P = 128
NCH = N // P             # 32 row-chunks of the score matrix
MT = 512                 # m-tile width (PSUM free dim)
NMT = MLOC // MT         # 4 m-tiles per core
GRP = 3                  # score chunks exp'd per ScalarE instruction
CP1 = C + 1              # v columns + ones column
VP = 80                  # padded v chunk stride (bytes, %16==0) for DoubleRow LDW
EXP_BIAS = -1.3862943611198906  # -ln(4): cancels in num/den, keeps e' in fp8 range

F32 = mybir.dt.float32
BF16 = mybir.dt.bfloat16
FP16 = mybir.dt.float16
F8 = mybir.dt.float8e4
EXP = mybir.ActivationFunctionType.Exp
DROW = mybir.MatmulPerfMode.DoubleRow

_NC_CACHE = {}


def _build(grp=GRP, spsum_bufs=2, exp_bufs=2, prec="fp8av", ilv=True,
           loop_reps=None):
    """Build the per-core graph.

    grp: score chunks per exp instruction ([128, grp*512] PSUM group).
    spsum_bufs: score-PSUM group buffers (grp*spsum_bufs + 2 <= 8 banks).
    exp_bufs: exp_sb SBUF buffers.
    prec: "fp16" (all fp16, PE ~131072 cyc/iter) or "fp8av" (fp16 scores,
        fp8e4 exp/v with DoubleRow AV, PE ~103k cyc/iter).
    ilv: interleave AV matmuls of m-tile t-1 between score groups of
        m-tile t (keeps the PE busy while ACT catches up on exp).
    loop_reps: if set, wrap the attention body in a hardware For_i loop
        (used only for timing: per-iteration time = slope over reps).
    """
    assert grp * spsum_bufs + 2 <= 8
    fp8 = prec == "fp8av"
    qk_dt = FP16
    ev_dt = F8 if fp8 else {"fp16": FP16, "bf16": BF16}[prec]
    vp = VP if fp8 else CP1
    nc = bacc.Bacc("TRN2", target_bir_lowering=False, debug=False)
    q_ext = nc.declare_dram_parameter("q", [C, N], qk_dt, isOutput=False)
    k_ext = nc.declare_dram_parameter("k", [C, MLOC], qk_dt, isOutput=False)
    v_ext = nc.declare_dram_parameter("v", [P, NCH, vp], ev_dt, isOutput=False)
    out_ext = nc.declare_dram_parameter("out", [CP1, MLOC], F32, isOutput=True)

    # n-chunk groups per m-tile, e.g. grp=3 -> [3]*10 + [2]
    gsizes = []
    left = NCH
    while left > 0:
        gsizes.append(min(grp, left))
        left -= gsizes[-1]

    with tile.TileContext(nc) as tc:
        with (
            tc.tile_pool(name="const", bufs=1) as cpool,
            tc.tile_pool(name="expp", bufs=exp_bufs) as epool,
            tc.tile_pool(name="outp", bufs=2) as opool,
            tc.tile_pool(name="spsum", bufs=spsum_bufs, space="PSUM") as spool,
            tc.tile_pool(name="apsum", bufs=2, space="PSUM") as apool,
        ):
            # One serial HWDGE queue -> emit in first-needed order: the first
            # scores group needs q[:, :384] and k[:, :512]; v is needed a few
            # us in (first AV matmul); later k/q chunks are consumed later.
            k_sb = cpool.tile([C, MLOC], qk_dt)
            q_sb = cpool.tile([C, N], qk_dt)
            v_sb = cpool.tile([P, NCH, vp], ev_dt)
            bias_sb = cpool.tile([P, 1], F32)
            nc.gpsimd.memset(bias_sb[:], EXP_BIAS)

            def dq(j):
                nc.sync.dma_start(
                    q_sb[:, j * 512:(j + 1) * 512], q_ext[:, j * 512:(j + 1) * 512]
                )

            def dk(j):
                nc.sync.dma_start(
                    k_sb[:, j * 512:(j + 1) * 512], k_ext[:, j * 512:(j + 1) * 512]
                )

            def dv(j):
                nc.sync.dma_start(
                    v_sb[:, j * 8:(j + 1) * 8, :], v_ext[:, j * 8:(j + 1) * 8, :]
                )

            dq(0); dk(0); dq(1); dv(0); dq(2); dv(1); dq(3); dv(2)
            dq(4); dv(3); dq(5); dq(6); dq(7); dk(1); dk(2); dk(3)

            def q_ap(i):
                return q_sb[:, i * P:(i + 1) * P]

            def k_ap(t):
                return k_sb[:, t * MT:(t + 1) * MT]

            def av_units(t, exp_sb, pav):
                """AV matmul emitters for m-tile t (accumulate into pav)."""
                if fp8:
                    npair = NCH // 2

                    def mk(i):
                        def emit():
                            nc.tensor.matmul(
                                pav[:],
                                lhsT=v_sb[:, 2 * i:2 * i + 2, :CP1],
                                rhs=exp_sb[:, 2 * i:2 * i + 2, :],
                                start=(i == 0),
                                stop=(i == npair - 1),
                                perf_mode=DROW,
                            )
                        return emit

                    return [mk(i) for i in range(npair)]

                def mk(i):
                    def emit():
                        nc.tensor.matmul(
                            pav[:],
                            lhsT=v_sb[:, i, :CP1],
                            rhs=exp_sb[:, i, :],
                            start=(i == 0),
                            stop=(i == NCH - 1),
                        )
                    return emit

                return [mk(i) for i in range(NCH)]

            def finish_mtile(t, pav):
                o_sb = opool.tile([CP1, MT], F32, tag="ot")
                nc.vector.tensor_copy(o_sb[:], pav[:])
                nc.sync.dma_start(out_ext[:, t * MT:(t + 1) * MT], o_sb[:])

            def attention_body(iv=None):
                prev = None  # (t-1, its pending AV units, its pav)
                for t in range(NMT):
                    exp_sb = epool.tile([P, NCH, MT], ev_dt, tag="exp")
                    i = 0
                    ng = len(gsizes)
                    for g, gs in enumerate(gsizes):
                        ps = spool.tile([P, grp, MT], F32, tag="sc")
                        for u in range(gs):
                            nc.tensor.matmul(
                                ps[:, u, :],
                                lhsT=q_ap(i + u),
                                rhs=k_ap(t),
                                start=True,
                                stop=True,
                            )
                        nc.scalar.activation(
                            exp_sb[:, i:i + gs, :], ps[:, :gs, :], EXP,
                            bias=bias_sb[:] if fp8 else 0.0,
                        )
                        i += gs
                        if ilv and prev is not None:
                            pt, units, nu, ppav = prev
                            take = (g + 1) * nu // ng - g * nu // ng
                            for _ in range(take):
                                units.pop(0)()
                            if g == ng - 1:
                                assert not units
                                finish_mtile(pt, ppav)
                    pav = apool.tile([CP1, MT], F32, tag="av")
                    units = av_units(t, exp_sb, pav)
                    if ilv:
                        prev = (t, units, len(units), pav)
                    else:
                        for emit in units:
                            emit()
                        finish_mtile(t, pav)
                if ilv and prev is not None:
                    pt, units, nu, ppav = prev
                    for emit in units:
                        emit()
                    finish_mtile(pt, ppav)

            if loop_reps is None:
                attention_body()
            else:
                with tc.For_i(0, loop_reps, 1):
                    attention_body()

    nc.compile()
    return nc


BEST = {"grp": 3, "spsum_bufs": 2, "prec": "fp8av", "ilv": True}


def _get_nc():
    if "nc" not in _NC_CACHE:
        _NC_CACHE["nc"] = _build(**BEST)
    return _NC_CACHE["nc"]


def _make_in_maps(x, Wq, Wk, Wv, prec="fp8av"):
    fp8 = prec == "fp8av"
    ev_np = ml_dtypes.float8_e4m3fn if fp8 else (
        np.float16 if prec == "fp16" else ml_dtypes.bfloat16)
    vp = VP if fp8 else CP1
    x = np.asarray(x, np.float32)
    wq8 = np.asarray(Wq, np.float32) * 0.125
    wk = np.asarray(Wk, np.float32)
    wv = np.asarray(Wv, np.float32)
    in_maps = []
    for b in range(B):
        xb = x[b]                                  # [C, N]
        qt = np.ascontiguousarray(wq8 @ xb)        # [C, N]
        kf = wk @ xb                               # [C, N]
        vt = (wv @ xb).T                           # [N, C]
        va = np.zeros((P, NCH, vp), np.float32)
        v3 = vt.reshape(NCH, P, C).transpose(1, 0, 2)   # [P, NCH, C]
        va[:, :, :C] = v3
        va[:, :, C] = 1.0
        va = va.astype(ev_np)
        for h in range(2):
            in_maps.append(
                {
                    "q": qt.astype(np.float16),
                    "k": np.ascontiguousarray(
                        kf[:, h * MLOC:(h + 1) * MLOC]
                    ).astype(np.float16),
                    "v": va,
                }
            )
    return in_maps


def _assemble(results):
    out = np.empty((B, C, N), np.float32)
    for core in range(2 * B):
        b, h = divmod(core, 2)
        r = results[core]["out"]
        out[b, :, h * MLOC:(h + 1) * MLOC] = r[:C] / r[C:C + 1]
    return out


def run(x, Wq, Wk, Wv, trace=False, **trace_kwargs):
    nc = _get_nc()
    res = run_bass_kernel_spmd(
        nc,
        _make_in_maps(x, Wq, Wk, Wv, prec=BEST.get("prec", "fp8av")),
        core_ids=list(range(2 * B)),
        trace=trace,
        **trace_kwargs,
    )
    return _assemble(res.results), res


def kernel(x, Wq, Wk, Wv):
    out, _ = run(x, Wq, Wk, Wv, trace=False)
    return out
